# revision 1
# baseline (speedup 1.0000x reference)
"""Trainium2 Bass kernel for nn_CLEAR_45561013076524 (vq_codebook).

Pure data-parallel over 8 NeuronCores: each core computes 512 images of the
conv-encoder -> SoftSOM -> (collapsed) classifier pipeline.

Mathematical simplifications (validated numerically against the reference):
  * The node-attention block has n_nodes=1, so its softmax is identically 1
    and `fused == blended` tiled 4x.  Therefore
       logits = blended @ sum_h clf_w[h*256:(h+1)*256] + clf_b
    and y/class_emb/query_*/attn_*/node_emb are dead inputs.
  * conv1 (5x5, stride 2, pad 1) is one K=75 im2col matmul (host-built
    patches), zero-padded to K=128 so the PE stays at its full 2.4 GHz
    p-state (K<128 throttles the array to 1.2 GHz, measured).
  * cdist^2 is computed as one accumulated matmul chain by augmenting the
    contraction with ones/|z|^2 rows against |c|^2/ones columns and
    pre-scaling c^T by -2.

Matmul convention: out[M,N] = lhsT[K,M].T @ rhs[K,N], K on SBUF partitions.

Perf structure (all measured on HW via microbenchmarks):
  * The PE throttles to 1.2 GHz whenever fewer than ~128 array rows are
    active, so every matmul in the hot path is arranged to keep all 128
    rows busy (zero-padding K and zero-masked weight rows where needed).
  * conv1 lhsT is [128,128] = 4 column-stacked copies of the weights, so the
    single matmul also materializes 4 replicas of h1 across the 4 partition
    quadrants -- free input replication for conv2's row-tiling.
  * conv2/conv3: the 9 taps run on 2 concurrent 64-row PE tiles via
    tile_position=(64i,0) (conv2's K=32 zero-padded to 64), accumulating
    into 2 PSUM banks that are merged at eviction (ACT copy + DVE add +
    DVE fused bias-relu, chosen to balance the two PSUM-capable engines).
  * conv4 is K=128/M=128/N=512 at full-clock cadence (~248ns incl. the
    3-dim access-pattern walker overhead).
  * enc runs "swapped": h4 column tiles are the stationary operand and
    enc_w streams as the moving operand with N=256, so the per-matmul
    weight load hides under the matmul; the phase is enc_w-DMA-bound.
  * The SoftSOM tail is split into distance / softmax / transpose passes so
    the in-order PE never stalls behind the ACT/DVE softmax chain.
"""

import numpy as np
import ml_dtypes

import concourse.bass as bass
from concourse import bacc
from concourse import mybir
from concourse.tile import TileContext
from concourse.bass_utils import run_bass_kernel_spmd
from concourse.masks import make_identity

BF16NP = ml_dtypes.bfloat16
F32 = mybir.dt.float32
F32R = mybir.dt.float32r
BF = mybir.dt.bfloat16
F8 = mybir.dt.float8e4
AF = mybir.ActivationFunctionType
ALU = mybir.AluOpType

NCORES = 8
B = 4096
BL = B // NCORES          # images per core
CH = 64                   # chunk (images) for conv3/conv4
SUB = 32                  # sub-chunk for conv1/conv2
PAIR = 2 * CH             # images per enc pass

OFF9 = [(ky, kx) for ky in range(3) for kx in range(3)]

_CACHE = {}


# --------------------------------------------------------------------------
# host-side input preparation (layout only / tiny parameter math)
# --------------------------------------------------------------------------

def _prep_host(inputs):
    f32 = np.float32
    x = np.ascontiguousarray(np.asarray(inputs['x'], f32))
    xp = np.zeros((B, 3, 34, 34), f32)
    xp[:, :, 1:33, 1:33] = x
    # conv1 im2col on host (pure gather): xim[b, (ci,ky,kx), (oy,ox)]
    from numpy.lib.stride_tricks import sliding_window_view
    win = sliding_window_view(xp, (5, 5), axis=(2, 3))[:, :, ::2, ::2]
    xim = np.zeros((B, 128, 225), BF16NP)   # K pre-padded to 128 rows
    xim[:, :75] = win.transpose(0, 1, 4, 5, 2, 3).reshape(
        B, 75, 225).astype(BF16NP)

    c1w = np.asarray(inputs['conv1_w'], f32)
    w1 = c1w.transpose(1, 2, 3, 0).reshape(75, 32)
    w1p = np.zeros((128, 128), f32)           # K padded to 128, M tiled 4x
    for g in range(4):
        w1p[:75, 32 * g:32 * g + 32] = w1
    w1p = w1p.astype(BF16NP)
    b1r = np.tile(np.asarray(inputs['conv1_b'], f32), 4).reshape(128, 1)

    def conv_lhsT(w):  # [CO,CI,3,3] -> [CI, 9, CO]
        return np.ascontiguousarray(
            w.transpose(1, 2, 3, 0).reshape(w.shape[1], 9, w.shape[0]))

    # conv2: weights column-stacked 2x (to emit 2 replicas of h2 for conv3's
    # row tiling), zero-padded from K=32 to K=64 (so both 64-row PE tiles
    # keep all 128 array rows active -> full clock), 2x row-tiled.
    w2 = conv_lhsT(np.asarray(inputs['conv2_w'], f32))      # [32, 9, 64]
    w2d = np.concatenate([w2, w2], axis=2)                  # [32, 9, 128]
    w2p = np.concatenate([w2d, np.zeros_like(w2d)], axis=0)  # [64, 9, 128]
    w2r = np.ascontiguousarray(np.tile(w2p, (2, 1, 1))).astype(BF16NP)
    b2r = np.tile(np.asarray(inputs['conv2_b'], f32), 2).reshape(128, 1)

    w3 = conv_lhsT(np.asarray(inputs['conv3_w'], f32))      # [64, 9, 128]
    w3r = np.ascontiguousarray(np.tile(w3, (2, 1, 1))).astype(BF16NP)
    b3r = np.asarray(inputs['conv3_b'], f32).reshape(128, 1)

    w4f = np.asarray(inputs['conv4_w'], f32)                # [256,128,3,3]
    w4 = np.ascontiguousarray(
        w4f.reshape(2, 128, 128, 3, 3).transpose(2, 3, 4, 0, 1)
        .reshape(128, 9, 2, 128)).astype(BF16NP)
    b4 = np.ascontiguousarray(
        np.asarray(inputs['conv4_b'], f32).reshape(2, 128).T)  # [128,2]

    # enc (swapped): moving operand encw_m[ct, y, c, x, dout]
    ew = np.asarray(inputs['enc_w'], f32).reshape(2, 128, 8, 8, 256)
    encw = np.ascontiguousarray(ew.transpose(0, 2, 1, 3, 4)).astype(BF16NP)
    encb = np.broadcast_to(np.asarray(inputs['enc_b'], f32), (128, 256)).copy()

    nf = np.asarray(inputs['node_fc_w'], f32).reshape(2, 128, 2, 128)
    nfc = np.ascontiguousarray(nf.transpose(1, 0, 2, 3))       # [k,kt,mt,m]
    nfcb = np.ascontiguousarray(
        np.asarray(inputs['node_fc_b'], f32).reshape(2, 128).T)

    protos = np.asarray(inputs['protos'], f32)
    grid = np.asarray(inputs['grid_pos'], f32)

    def dist_rhs(c):
        # rp[k, kt, n] = -2*c[n, kt*128+k]
        # aug (K padded to 128 to keep the PE p-state up):
        #   row0 = |c|^2 (pairs with the all-ones lhsT row)
        #   row1 = ones  (pairs with the |z|^2 lhsT row)
        rp = np.ascontiguousarray(
            (-2.0 * c.T).reshape(2, 128, 256).transpose(1, 0, 2))
        aug = np.zeros((128, 256), f32)
        aug[0] = (c * c).sum(1)
        aug[1] = 1.0
        return rp.astype(f32), aug.astype(f32)

    rp, rpa = dist_rhs(protos)
    rg, rga = dist_rhs(grid)

    # blended is only consumed by the classifier, so fold protos into it:
    # logits = w_norm @ (protos @ clf_sum) + clf_b
    clf_sum = np.asarray(inputs['clf_w'], f32).reshape(4, 256, 10).sum(0)
    pc = (protos.astype(np.float64) @ clf_sum.astype(np.float64)).astype(f32)
    clfs = np.ascontiguousarray(
        pc.reshape(2, 128, 10).transpose(1, 0, 2))             # [128, 2, 10]
    clfb = np.broadcast_to(np.asarray(inputs['clf_b'], f32), (128, 10)).copy()

    gate = 1.0 / (1.0 + np.exp(-np.asarray(inputs['gate_logits'], np.float64)))
    gateb = np.broadcast_to(gate.astype(f32), (128, 256)).copy()

    traw = float(np.asarray(inputs['temp_raw']).reshape(-1)[0])
    temp = 1.0 / (1.0 + np.exp(-traw)) * (1.0 - 0.001) + 0.001
    invt = np.full((128, 1), 1.0 / temp, f32)
    ninvt = np.full((128, 1), -1.0 / temp, f32)

    shared = dict(w1=w1p, w2=w2r, w3=w3r, w4=w4,
                  onesr=np.ones((1, 512), f32), onescol=np.ones((128, 1), f32),
                  b1=b1r, b2=b2r, b3=b3r,
                  b4=b4, encw=encw, encb=encb, nfc=nfc, nfcb=nfcb,
                  rp=rp, rpa=rpa, rg=rg, rga=rga,
                  clfs=clfs, clfb=clfb, gateb=gateb, invt=invt, ninvt=ninvt)
    return xim, shared


# --------------------------------------------------------------------------
# device program
# --------------------------------------------------------------------------

def _build_nc():
    nc = bacc.Bacc(None, target_bir_lowering=False)
    P = nc.declare_dram_parameter
    xim = P("xim", [BL, 128, 225], BF, isOutput=False)
    w1 = P("w1", [128, 128], BF, isOutput=False)
    w2 = P("w2", [128, 9, 128], BF, isOutput=False)
    w3 = P("w3", [128, 9, 128], BF, isOutput=False)
    w4 = P("w4", [128, 9, 2, 128], BF, isOutput=False)
    b1 = P("b1", [128, 1], F32, isOutput=False)
    b2 = P("b2", [128, 1], F32, isOutput=False)
    b3 = P("b3", [128, 1], F32, isOutput=False)
    b4 = P("b4", [128, 2], F32, isOutput=False)
    encw = P("encw", [2, 8, 128, 8, 256], BF, isOutput=False)
    encb = P("encb", [128, 256], F32, isOutput=False)
    nfc = P("nfc", [128, 2, 2, 128], F32, isOutput=False)
    nfcb = P("nfcb", [128, 2], F32, isOutput=False)
    rp = P("rp", [128, 2, 256], F32, isOutput=False)
    rpa = P("rpa", [128, 256], F32, isOutput=False)
    rg = P("rg", [128, 2, 256], F32, isOutput=False)
    rga = P("rga", [128, 256], F32, isOutput=False)
    clfs = P("clfs", [128, 2, 10], F32, isOutput=False)
    clfb = P("clfb", [128, 10], F32, isOutput=False)
    gateb = P("gateb", [128, 256], F32, isOutput=False)
    invt = P("invt", [128, 1], F32, isOutput=False)
    onesr = P("onesr", [1, 512], F32, isOutput=False)
    onescol = P("onescol", [128, 1], F32, isOutput=False)
    ninvt = P("ninvt", [128, 1], F32, isOutput=False)
    outd = P("out", [BL, 10], F32, isOutput=True)

    with TileContext(nc) as tc:
        with (tc.tile_pool(name="consts", bufs=1) as consts,
              tc.tile_pool(name="acts", bufs=1) as acts,
              tc.tile_pool(name="encwp", bufs=6) as encwp,
              tc.tile_pool(name="cvtmp", bufs=6) as cvtmp,
              tc.tile_pool(name="smp", bufs=3) as smp,
              tc.tile_pool(name="stats", bufs=8) as stats,
              tc.tile_pool(name="outp", bufs=2) as outp,
              tc.tile_pool(name="psA", bufs=6, space="PSUM") as psA,
              tc.tile_pool(name="psB", bufs=2, space="PSUM") as psB):

            dma = nc.sync.dma_start

            # ---- conv1-critical loads first (everything else overlaps) ----
            w1s = consts.tile([128, 128], BF); dma(out=w1s, in_=w1[:])
            b1s = consts.tile([128, 1], F32); dma(out=b1s, in_=b1[:])
            pts = []
            for i in range(2):
                t = acts.tile([128, SUB, 15, 15], BF, name=f"pt{i}")
                pts.append(t)

            def load_patches(b0, pt):
                base = xim[b0, 0, 0]
                src = bass.AP(
                    tensor=base.tensor, offset=base.offset,
                    ap=[[225, 128], [128 * 225, SUB], [1, 225]])
                dma(out=pt[:], in_=src)

            load_patches(0, pts[0])

            # ---- remaining constants --------------------------------------
            w2s = consts.tile([128, 9, 128], BF); dma(out=w2s, in_=w2[:])
            w3s = consts.tile([128, 9, 128], BF); dma(out=w3s, in_=w3[:])
            w4s = consts.tile([128, 9, 2, 128], BF); dma(out=w4s, in_=w4[:])
            b2s = consts.tile([128, 1], F32); dma(out=b2s, in_=b2[:])
            b3s = consts.tile([128, 1], F32); dma(out=b3s, in_=b3[:])
            b4s = consts.tile([128, 2], F32); dma(out=b4s, in_=b4[:])
            encbs = consts.tile([128, 256], F32); dma(out=encbs, in_=encb[:])
            ident = consts.tile([128, 128], F32)
            make_identity(nc, ident)

            # ---- persistent activation tensors ----------------------------
            # conv1 patches: K zero-padded to 128; two buffers alternate.
            h1p4 = acts.tile([128, SUB, 17, 17], BF)   # 4 replicas of h1
            nc.vector.memset(h1p4, 0.0)
            h2p2 = acts.tile([128, CH, 10, 10], BF)    # 2 replicas of h2
            nc.gpsimd.memset(h2p2, 0.0)
            h3p = acts.tile([128, CH, 10, 10], BF)
            nc.gpsimd.memset(h3p, 0.0)
            h4t = acts.tile([128, 2, PAIR, 8, 8], BF)  # [c, ct, b, y, x]
            z0b = acts.tile([128, 4, 256], F32)        # [b, pair, dout]
            z0T = acts.tile([128, 2, BL], F32)
            zT = acts.tile([128, 2, BL], F32)
            wT = acts.tile([128, 2, BL], F32)

            # ---- conv/enc pipeline over image chunks ----------------------
            for c in range(BL // CH):
                pb = (c % 2) * CH
                for s in range(CH // SUB):
                    b0 = c * CH + s * SUB
                    # conv1: single K=128(padded) matmul per image pair; the
                    # 4x column-stacked weights emit 4 replicas of h1.
                    pt = pts[(2 * c + s) % 2]
                    if not (c == 0 and s == 0):
                        load_patches(b0, pt)
                    for j in range(SUB // 2):
                        ps1 = psA.tile([128, 2, 15, 15], F32, tag="ps")
                        nc.tensor.matmul(ps1[:], w1s[:], pt[:, 2 * j:2 * j + 2],
                                         start=True, stop=True)
                        dst1 = h1p4[:, 2 * j:2 * j + 2, 1:16, 1:16]
                        if j % 2 == 0:
                            nc.scalar.activation(out=dst1, in_=ps1[:],
                                                 func=AF.Relu,
                                                 bias=b1s[:, 0:1])
                        else:
                            nc.vector.tensor_scalar(
                                out=dst1, in0=ps1[:], scalar1=b1s[:, 0:1],
                                scalar2=0.0, op0=ALU.add, op1=ALU.max)

                    # conv2: 9 taps on 2 concurrent 64-row PE tiles (K
                    # zero-padded 32->64 so all 128 array rows stay active
                    # and the PE holds its full 2.4 GHz p-state).
                    for ct in range(4):
                        pc2 = [psA.tile([128, 512], F32, tag="ps",
                                        name=f"pc2_{i}") for i in range(2)]
                        ib = s * SUB + ct * 8
                        for k, (ky, kx) in enumerate(OFF9):
                            i = k % 2
                            rhs = h1p4[64 * i:64 * i + 64,
                                       ct * 8:(ct + 1) * 8,
                                       ky:ky + 15:2, kx:kx + 15:2]
                            nc.tensor.matmul(
                                pc2[i][:], w2s[64 * i:64 * i + 64, k], rhs,
                                start=(k < 2), stop=(k >= 7),
                                tile_position=(64 * i, 0))
                        t3 = cvtmp.tile([128, 512], BF)
                        nc.scalar.activation(out=t3, in_=pc2[0][:],
                                             func=AF.Copy)
                        nc.vector.tensor_add(t3, t3, pc2[1][:])
                        nc.vector.tensor_scalar(
                            out=h2p2[:, ib:ib + 8, 1:9, 1:9],
                            in0=t3.rearrange("p (b y x) -> p b y x", b=8, y=8),
                            scalar1=b2s[:, 0:1], scalar2=0.0,
                            op0=ALU.add, op1=ALU.max)

                # conv3: 9 taps on 2 concurrent 64-row PE tiles
                for ct in range(8):
                    pc3 = [psA.tile([128, 512], F32, tag="ps",
                                    name=f"pc3_{i}") for i in range(2)]
                    for k, (ky, kx) in enumerate(OFF9):
                        i = k % 2
                        rhs = h2p2[64 * i:64 * i + 64, ct * 8:(ct + 1) * 8,
                                   ky:ky + 8, kx:kx + 8]
                        nc.tensor.matmul(
                            pc3[i][:], w3s[64 * i:64 * i + 64, k], rhs,
                            start=(k < 2), stop=(k >= 7),
                            tile_position=(64 * i, 0))
                    t1 = cvtmp.tile([128, 512], BF)
                    nc.scalar.activation(out=t1, in_=pc3[0][:], func=AF.Copy)
                    nc.vector.tensor_add(t1, t1, pc3[1][:])
                    nc.vector.tensor_scalar(
                        out=h3p[:, ct * 8:(ct + 1) * 8, 1:9, 1:9],
                        in0=t1.rearrange("p (b y x) -> p b y x", b=8, y=8),
                        scalar1=b3s[:, 0:1], scalar2=0.0,
                        op0=ALU.add, op1=ALU.max)

                # conv4: K=128, two co-tiles; evict into [c, ct, y, x, b]
                for mt in range(2):
                    done = 0
                    while done < 8:
                        g = min(6, 8 - done)
                        pc4 = [psA.tile([128, 8, 8, 8], F32, tag="ps",
                                        name=f"pc4_{i}") for i in range(g)]
                        for k, (ky, kx) in enumerate(OFF9):
                            lhsT = w4s[:, k, mt]
                            for i in range(g):
                                t = done + i
                                rhs = h3p[:, t * 8:(t + 1) * 8,
                                          ky:ky + 8, kx:kx + 8]
                                nc.tensor.matmul(pc4[i][:], lhsT, rhs,
                                                 start=(k == 0), stop=(k == 8))
                        for i in range(g):
                            t = done + i
                            dst = h4t[:, mt, pb + t * 8:pb + (t + 1) * 8]
                            if t % 2 == 0:
                                nc.scalar.activation(
                                    out=dst, in_=pc4[i][:],
                                    func=AF.Relu, bias=b4s[:, mt:mt + 1])
                            else:
                                nc.vector.tensor_scalar(
                                    out=dst, in0=pc4[i][:],
                                    scalar1=b4s[:, mt:mt + 1], scalar2=0.0,
                                    op0=ALU.add, op1=ALU.max)
                        done += g

                # enc (swapped): h4 column-tiles stationary, enc_w moving
                if c % 2 == 1:
                    p = c // 2
                    zp = psB.tile([128, 256], F32, tag="pe")
                    for ct in range(2):
                        for y in range(8):
                            ewt = encwp.tile([128, 8, 256], BF)
                            nc.gpsimd.dma_start(out=ewt, in_=encw[ct, y])
                            for xx in range(8):
                                first = (ct == 0 and y == 0 and xx == 0)
                                last = (ct == 1 and y == 7 and xx == 7)
                                nc.tensor.matmul(
                                    zp[:], h4t[:, ct, :, y, xx], ewt[:, xx],
                                    start=first, stop=last)
                    nc.vector.tensor_add(z0b[:, p], zp[:], encbs)
                    # transpose this pair's z0 into z0T right away so it
                    # overlaps with the next chunks' conv work
                    for kt in range(2):
                        tp = psA.tile([128, 128], F32, tag="ps")
                        nc.tensor.transpose(
                            tp[:], z0b[:, p, 128 * kt:128 * kt + 128],
                            ident[:])
                        nc.vector.tensor_copy(
                            out=z0T[:, kt, p * 128:(p + 1) * 128], in_=tp[:])

            # softsom constants -- loaded late so their DMAs overlap the
            # conv pipeline instead of delaying its first matmul
            nfcs = consts.tile([128, 2, 2, 128], F32); dma(out=nfcs, in_=nfc[:])
            nfcbs = consts.tile([128, 2], F32); dma(out=nfcbs, in_=nfcb[:])
            rps = consts.tile([128, 2, 256], F32); dma(out=rps, in_=rp[:])
            rpas = consts.tile([128, 256], F32); dma(out=rpas, in_=rpa[:])
            rgs = consts.tile([128, 2, 256], F32); dma(out=rgs, in_=rg[:])
            rgas = consts.tile([128, 256], F32); dma(out=rgas, in_=rga[:])
            clfss = consts.tile([128, 2, 10], F32); dma(out=clfss, in_=clfs[:])
            clfbs = consts.tile([128, 10], F32); dma(out=clfbs, in_=clfb[:])
            gatebs = consts.tile([128, 256], F32); dma(out=gatebs, in_=gateb[:])
            invts = consts.tile([128, 1], F32); dma(out=invts, in_=invt[:])
            ninvts = consts.tile([128, 1], F32); dma(out=ninvts, in_=ninvt[:])
            ones_col = consts.tile([128, 1], F32)
            dma(out=ones_col, in_=onescol[:])
            z2row = consts.tile([1, BL], F32)    # |z|^2 per image
            aug2 = consts.tile([128, BL], F32)   # K-padded aug lhsT
            nc.vector.memset(aug2, 0.0)
            dma(out=aug2[0:1], in_=onesr[:])

            # ---- SoftSOM head (fp32) --------------------------------------
            for mt in range(2):
                zp = psA.tile([128, BL], F32, tag="ps")
                for kt in range(2):
                    nc.tensor.matmul(zp[:], nfcs[:, kt, mt],
                                     z0T[:, kt],
                                     start=(kt == 0), stop=(kt == 1))
                nc.vector.tensor_scalar(out=zT[:, mt], in0=zp[:],
                                        scalar1=nfcbs[:, mt:mt + 1],
                                        scalar2=None, op0=ALU.add)

            zp2 = psA.tile([1, BL], F32, tag="ps")
            for kt in range(2):
                sqk = cvtmp.tile([128, 512], F32, tag='sqk', bufs=2)
                nc.scalar.activation(out=sqk, in_=zT[:, kt], func=AF.Square)
                nc.tensor.matmul(zp2[:], ones_col[:],
                                 sqk,
                                 start=(kt == 0), stop=(kt == 1))
            nc.vector.tensor_copy(out=z2row, in_=zp2[:])
            dma(out=aug2[1:2], in_=z2row)

            # pass 1: distances for all b-tiles (PE-heavy, pipelined)
            dts = []
            for bt in range(BL // 128):
                bs = slice(bt * 128, (bt + 1) * 128)
                parts = []
                for rmain, raug in ((rps, rpas), (rgs, rgas)):
                    dp = psA.tile([128, 256], F32, tag="ps", name=f"dp{bt}")
                    nc.tensor.matmul(dp[:], zT[:, 0, bs], rmain[:, 0],
                                     start=True, stop=False)
                    nc.tensor.matmul(dp[:], zT[:, 1, bs], rmain[:, 1],
                                     start=False, stop=False)
                    nc.tensor.matmul(dp[:], aug2[:, bs], raug[:],
                                     start=False, stop=True)
                    t = smp.tile([128, 256], F32, name=f"t{bt}", tag="sm",
                                 bufs=8)
                    nc.scalar.activation(out=t, in_=dp[:], func=AF.Relu)
                    nc.scalar.activation(out=t, in_=t, func=AF.Sqrt)
                    parts.append(t)
                dtot = smp.tile([128, 256], F32, name=f"dt{bt}", tag="dt",
                                bufs=4)
                nc.vector.tensor_add(dtot, parts[0], parts[1])
                dts.append(dtot)

            # pass 2: softmax chains (ACT/DVE only, no PE)
            wns = []
            for bt in range(BL // 128):
                dtot = dts[bt]
                mn = stats.tile([128, 1], F32)
                nc.vector.tensor_reduce(out=mn, in_=dtot,
                                        axis=mybir.AxisListType.X, op=ALU.min)
                mb = stats.tile([128, 1], F32)
                nc.vector.tensor_mul(mb, mn, invts)
                e = smp.tile([128, 256], F32, name=f"e{bt}", tag="e", bufs=2)
                s0 = stats.tile([128, 1], F32)
                nc.scalar.activation(out=e, in_=dtot, func=AF.Exp,
                                     bias=mb[:, 0:1], scale=ninvts[:, 0:1],
                                     accum_out=s0)
                eg = smp.tile([128, 256], F32, name=f"eg{bt}", tag="eg",
                              bufs=2)
                nc.vector.tensor_mul(eg, e, gatebs)
                s1 = stats.tile([128, 1], F32)
                nc.vector.tensor_reduce(out=s1, in_=eg,
                                        axis=mybir.AxisListType.X, op=ALU.add)
                t3 = stats.tile([128, 1], F32)
                nc.vector.tensor_scalar(out=t3, in0=s0, scalar1=1e-8,
                                        scalar2=None, op0=ALU.mult)
                den = stats.tile([128, 1], F32)
                nc.vector.tensor_add(den, s1, t3)
                wi = stats.tile([128, 1], F32)
                nc.vector.reciprocal(wi, den)
                wn = smp.tile([128, 256], F32, name=f"wn{bt}", tag="wn",
                              bufs=4)
                nc.vector.tensor_scalar(out=wn, in0=eg, scalar1=wi[:, 0:1],
                                        scalar2=None, op0=ALU.mult)
                wns.append(wn)

            # pass 3: transposes (PE)
            for bt in range(BL // 128):
                bs = slice(bt * 128, (bt + 1) * 128)
                for kt in range(2):
                    tp = psA.tile([128, 128], F32, tag="ps")
                    nc.tensor.transpose(
                        tp[:], wns[bt][:, kt * 128:(kt + 1) * 128], ident[:])
                    nc.vector.tensor_copy(out=wT[:, kt, bs], in_=tp[:])

            for bt in range(BL // 128):
                bs = slice(bt * 128, (bt + 1) * 128)
                lg = psA.tile([128, 10], F32, tag="ps")
                for kt in range(2):
                    nc.tensor.matmul(lg[:], wT[:, kt, bs], clfss[:, kt],
                                     start=(kt == 0), stop=(kt == 1))
                ot = outp.tile([128, 10], F32)
                nc.vector.tensor_add(ot, lg[:], clfbs)
                dma(out=outd[bt * 128:(bt + 1) * 128], in_=ot)

    nc.finalize()
    return nc


# --------------------------------------------------------------------------
# entry point
# --------------------------------------------------------------------------

def kernel(**inputs):
    xim, shared = _prep_host(inputs)
    if 'nc' not in _CACHE:
        _CACHE['nc'] = _build_nc()
    nc = _CACHE['nc']
    in_maps = []
    for c in range(NCORES):
        m = dict(shared)
        m['xim'] = np.ascontiguousarray(xim[c * BL:(c + 1) * BL])
        in_maps.append(m)
    res = run_bass_kernel_spmd(nc, in_maps, list(range(NCORES)))
    return np.concatenate([res.results[c]['out'] for c in range(NCORES)], 0)

